# revision 21
# baseline (speedup 1.0000x reference)
"""Trainium2 Bass kernel for a dense transformer block (B=4, N=1024, D=1024,
H=16, Dh=64, MLP 4x), distributed over 8 NeuronCores with ZERO collectives.

Sharding: core c handles batch b = c//2, sequence half = c%2 (512 query
rows).  K/V are computed for the batch's full 1024-token sequence on both
cores of a pair; the sequence is rotated per-core so the core's own 512 rows
are always rows 0..511 of its input, so all 8 cores run one identical SPMD
program.

v2 layout/scheduling notes:
- All weights are cast to bf16 AND pre-packed into their exact SBUF layouts
  on the HOST, so every weight DMA is a contiguous [128, N] blast and no
  on-chip casting is needed (v1 spent ~200us of engine time on casts and
  streamed 48MB of f32 weights through one DMA queue).
- x is passed twice: full-sequence bf16 (feeds LN1) and own-half f32 (the
  residual).  LN1 stats tolerate bf16 input easily at the 2e-2 gate.
- V (then Q) matmuls are interleaved into the LN1 loop so the PE is busy
  from ~5us on; K projection is pipelined per-head-pair against the
  attention inner loop so the ACT-bound softmax EXP (~0.7us per [128,512]
  tile, the real attention bottleneck) overlaps PE matmul work.
- Scores contract over Dh=64: the two heads of a d-tile run as two
  CONCURRENT K=64 matmuls in distinct row-groups of the PE array
  (tile_position (0,0)/(64,0)), halving score PE time.
- Softmax denominators ride in column 64 of V (ones column); the
  normalization reciprocal-broadcast runs on ACT+GPSIMD+DVE, not PE.
"""

import numpy as np

import bass_rust
import concourse.bass as bass
import concourse.mybir as mybir
import concourse.tile as tile
from concourse import library_config
from concourse.masks import make_identity

F32 = mybir.dt.float32
BF16 = mybir.dt.bfloat16
AF = mybir.ActivationFunctionType
ALU = mybir.AluOpType

P = 128
D = 1024
S = 1024          # full sequence (per batch)
SO = 512          # own rows per core
H = 16
DH = 64
F = 4096
EPS = 1e-5
N_CORES = 8

ND = D // P       # 8   d tiles
NS = S // P       # 8   full-seq tiles
NSO = SO // P     # 4   own-seq tiles
NF = F // P       # 32  ff tiles

# packed-consts block column offsets
OFF_LN1W, OFF_LN1B, OFF_LN2W, OFF_LN2B = 0, 8, 16, 24
OFF_BQ, OFF_BK, OFF_BFC = 32, 40, 48
OFF_BV, OFF_BO, OFF_BP = 80, 80 + 1024, 80 + 2048
NCONST = 80 + 3 * 1024


# --------------------------------------------------------------------------
# Workaround: this compiler build supports only ONE semaphore wait per
# instruction.  Move excess waits onto fresh NOPs inserted just before the
# offending instruction on the same engine.
# --------------------------------------------------------------------------
_counter = [0]


def _split_multiwaits(nc):
    nsplit = 0
    for fn in nc.m.functions:
        for blk in fn.blocks:
            il = list(blk.instructions)
            out = []
            changed = False
            for inst in il:
                si = inst.sync_info
                if si is not None and len(si.on_wait) > 1:
                    waits = list(si.on_wait)
                    for w in waits[:-1]:
                        _counter[0] += 1
                        nop = mybir.InstNoOp(
                            name=f"I-waitsplit-{_counter[0]}", ins=[], outs=[]
                        )
                        nop.engine = inst.engine
                        nop.sync_info = bass_rust.SyncInfo(on_wait=[w], on_update=[])
                        out.append(nop)
                        nc.register_instruction(nop, overwrite=True)
                    inst.sync_info = bass_rust.SyncInfo(
                        on_wait=[waits[-1]], on_update=list(si.on_update)
                    )
                    changed = True
                    nsplit += 1
                out.append(inst)
            if changed:
                blk.instructions = out
    return nsplit


def _vec_tile(nc, pool, ext, n):
    """Load a [n*128] dram vector as a [128, n] sbuf tile (col i = tile i)."""
    t = pool.tile([P, n], F32, name=ext.name + "_sb")
    nc.scalar.dma_start(out=t[:], in_=ext[:].rearrange("(o p) -> p o", p=P))
    return t


def _bcast_tile(nc, pool, ext, n):
    """Load a [n] dram vector broadcast to a [128, n] sbuf tile."""
    t = pool.tile([P, n], F32, name=ext.name + "_bc")
    ap = ext[:]
    src = bass.AP(tensor=ap.tensor, offset=ap.offset, ap=[[0, P], ap.ap[0]])
    nc.scalar.dma_start(out=t[:], in_=src)
    return t


def build():
    nc = bass.Bass(name="tfblock")

    xbf_ext = nc.declare_dram_parameter("x_bf", [S, D], BF16, isOutput=False)
    xo_ext = nc.declare_dram_parameter("x_own", [SO, D], F32, isOutput=False)
    wq_ext = nc.declare_dram_parameter("wq_p", [P, ND * D], BF16, isOutput=False)
    wk_ext = nc.declare_dram_parameter("wk_p", [P, ND * D], BF16, isOutput=False)
    wv_ext = nc.declare_dram_parameter("wv_p", [P, 2 * ND * SO], BF16, isOutput=False)
    wo_ext = nc.declare_dram_parameter("wo_p", [P, ND * D], BF16, isOutput=False)
    wfc_ext = nc.declare_dram_parameter("wfc_p", [P, NF * ND * P], BF16, isOutput=False)
    wp_ext = nc.declare_dram_parameter("wp_p", [P, NF * D], BF16, isOutput=False)
    cp_ext = nc.declare_dram_parameter("consts_p", [P, NCONST], F32, isOutput=False)
    out_ext = nc.declare_dram_parameter("out", [SO, D], F32, isOutput=True)

    def ln_tile(lnp, src_ap, hn_out, eps_t, tag):
        """LayerNorm stats on DVE + apply on ACT: hn_out = (src-mu)*rstd."""
        stats = lnp.tile([P, 2, 6], F32, tag=tag + "_st")
        for g in range(2):
            nc.vector.bn_stats(out=stats[:, g, :], in_=src_ap[:, g * 512 : (g + 1) * 512])
        mv = lnp.tile([P, 2], F32, tag=tag + "_mv")
        nc.vector.bn_aggr(out=mv[:], in_=stats[:])
        lnv = lnp.tile([P, 1], F32, tag=tag + "_sd")
        nc.scalar.activation(out=lnv[:], in_=mv[:, 1:2], func=AF.Ln, bias=eps_t[:])
        rstd = lnp.tile([P, 1], F32, tag=tag + "_rs")
        nc.scalar.activation(out=rstd[:], in_=lnv[:], func=AF.Exp, scale=-0.5)
        nb = lnp.tile([P, 1], F32, tag=tag + "_nb")
        nc.vector.tensor_scalar(nb[:], mv[:, 0:1], rstd[:], -1.0, ALU.mult, ALU.mult)
        nc.scalar.activation(
            out=hn_out, in_=src_ap, func=AF.Identity, bias=nb[:], scale=rstd[:]
        )

    with tile.TileContext(nc) as tc:
        from contextlib import ExitStack

        with ExitStack() as top:
            consts = top.enter_context(tc.tile_pool(name="consts", bufs=1))
            persist = top.enter_context(tc.tile_pool(name="persist", bufs=1))

            # all small consts arrive as ONE host-packed [128, 3152] f32
            # block on the ACT hwdge queue (tiny strided DMAs previously
            # jammed that queue for ~45us)
            cblk = consts.tile([P, NCONST], F32, name="cblk")
            def _col(off):
                return lambda i: cblk[:, off + i : off + i + 1]

            ln1w_c = _col(OFF_LN1W)
            ln1b_c = _col(OFF_LN1B)
            ln2w_c = _col(OFF_LN2W)
            ln2b_c = _col(OFF_LN2B)
            bq_c = _col(OFF_BQ)
            bk_c = _col(OFF_BK)
            bfc_c = _col(OFF_BFC)

            def bv_sl(a, b):
                return cblk[:, OFF_BV + a : OFF_BV + b]

            bo_bc = cblk[:, OFF_BO : OFF_BO + D]
            bp_bc = cblk[:, OFF_BP : OFF_BP + D]

            eps_t = consts.tile([P, 1], F32, name="eps")
            nc.vector.memset(eps_t[:], EPS)
            ident = consts.tile([P, P], BF16, name="ident")
            make_identity(nc, ident[:])
            e0 = consts.tile([P, P], BF16, name="e0")
            nc.vector.memset(e0[:], 0.0)
            nc.vector.memset(e0[0:1, :], 1.0)

            # residual stream (own half), f32, on the ACT queue (off the
            # critical startup path of the sync queue)
            xN_own = persist.tile([P, NSO, D], F32, name="xN_own")

            # long-lived mid tensors: Wo (consumed in the Wo projection),
            # OT (attention out, transposed), h2T (LN2 out, transposed)
            mid_cm = tc.tile_pool(name="mid", bufs=1)
            midp = mid_cm.__enter__()
            Wo_sb = midp.tile([P, ND, D], BF16, name="Wo_sb")
            OT = midp.tile([P, ND, SO], BF16, name="OT")
            h2T = midp.tile([P, ND, SO], BF16, name="h2T")

            # transposed-LN1 outputs + QKV live until end of attention
            hT_cm = tc.tile_pool(name="hTp", bufs=1)
            hTp = hT_cm.__enter__()
            hT_own = hTp.tile([P, ND, SO], BF16, name="hT_own")
            hT_oth = hTp.tile([P, ND, SO], BF16, name="hT_oth")

            qkv_cm = tc.tile_pool(name="qkvp", bufs=1)
            qkvp = qkv_cm.__enter__()
            QT = qkvp.tile([P, ND, SO], BF16, name="QT")
            KT = qkvp.tile([P, ND, S], BF16, name="KT")
            VN = qkvp.tile([P, NS, H, 66], BF16, name="VN")
            nc.vector.memset(VN[:, :, :, DH : DH + 1], 1.0)

            wk_cm = tc.tile_pool(name="wkp", bufs=1)
            wkp = wk_cm.__enter__()
            Wk_sb = wkp.tile([P, ND, D], BF16, name="Wk_sb")

            # ------------------------- phase B: x + W DMAs, LN1, V, Q
            with ExitStack() as phB:
                wv_pool = phB.enter_context(tc.tile_pool(name="wvp", bufs=1))
                xbf_pool = phB.enter_context(tc.tile_pool(name="xbfp", bufs=1))
                hnp = phB.enter_context(tc.tile_pool(name="hnp", bufs=3))
                lnp = phB.enter_context(tc.tile_pool(name="ln1", bufs=2))
                ps_t = phB.enter_context(tc.tile_pool(name="ps_t", bufs=2, space="PSUM"))
                ps_v = phB.enter_context(tc.tile_pool(name="ps_v", bufs=2, space="PSUM"))
                ps_q = phB.enter_context(tc.tile_pool(name="ps_q", bufs=2, space="PSUM"))

                Wv_sb = wv_pool.tile([P, 2, ND, SO], BF16, name="Wv_sb")
                Wq_sb = wv_pool.tile([P, ND, D], BF16, name="Wq_sb")
                xbf = xbf_pool.tile([P, NS, D], BF16, name="xbf")

                # sync-queue DMA order is the startup schedule: x tiles for
                # the own half, first V weight half, rest of x, V second
                # half, then Wq/Wk/Wo.
                # ONE fast queue, explicitly ordered: ln consts, own-half x,
                # remaining consts, Wv halves, other-half x, Wq, Wk, Wo.
                nc.sync.dma_start(out=cblk[:, 0:OFF_BV], in_=cp_ext[:, 0:OFF_BV])
                for st in range(4):
                    nc.sync.dma_start(
                        out=xbf[:, st, :], in_=xbf_ext[st * P : (st + 1) * P, :]
                    )
                nc.sync.dma_start(
                    out=cblk[:, OFF_BV:], in_=cp_ext[:, OFF_BV:]
                )
                nc.sync.dma_start(
                    out=Wv_sb[:, 0, :, :],
                    in_=wv_ext[:, 0 : ND * SO].rearrange("p (k c) -> p k c", k=ND),
                )
                nc.sync.dma_start(
                    out=Wv_sb[:, 1, :, :],
                    in_=wv_ext[:, ND * SO :].rearrange("p (k c) -> p k c", k=ND),
                )
                for st in range(4, NS):
                    nc.sync.dma_start(
                        out=xbf[:, st, :], in_=xbf_ext[st * P : (st + 1) * P, :]
                    )
                nc.sync.dma_start(
                    out=Wq_sb[:], in_=wq_ext[:].rearrange("p (k c) -> p k c", k=ND)
                )
                nc.sync.dma_start(
                    out=Wk_sb[:], in_=wk_ext[:].rearrange("p (k c) -> p k c", k=ND)
                )
                nc.sync.dma_start(
                    out=Wo_sb[:], in_=wo_ext[:].rearrange("p (k c) -> p k c", k=ND)
                )
                nc.scalar.dma_start(
                    out=xN_own[:], in_=xo_ext[:].rearrange("(t p) d -> p t d", p=P)
                )

                def emit_v(st, oh):
                    hTx = hT_own if st < 4 else hT_oth
                    st4 = st % 4
                    ps = ps_v.tile([P, SO], F32, tag="ps_v")
                    for kt in range(ND):
                        nc.tensor.matmul(
                            ps[:],
                            hTx[:, kt, st4 * P : (st4 + 1) * P],
                            Wv_sb[:, oh, kt, :],
                            start=(kt == 0),
                            stop=(kt == ND - 1),
                        )
                    nc.vector.tensor_tensor(
                        VN[:, st, oh * 8 : (oh + 1) * 8, 0:DH],
                        ps[:].rearrange("p (h e) -> p h e", h=8),
                        bv_sl(oh * 512, (oh + 1) * 512).rearrange(
                            "p (h e) -> p h e", h=8
                        ),
                        ALU.add,
                    )

                # LN1 per tile; h^T PE-transposes (ln1 w/b fused in the DVE
                # copy-back) run one tile ahead of the V matmuls
                for st in range(NS):
                    hn = hnp.tile([P, D], BF16, tag="hn")
                    ln_tile(lnp, xbf[:, st, :], hn[:], eps_t, "l1")
                    hTx = hT_own if st < 4 else hT_oth
                    st4 = st % 4
                    pst = ps_t.tile([P, ND, P], BF16, tag="ps_t")
                    for dt in range(ND):
                        nc.tensor.transpose(
                            pst[:, dt, :], hn[:, dt * P : (dt + 1) * P], ident[:]
                        )
                        if dt % 2 == 0:
                            nc.vector.tensor_scalar(
                                hTx[:, dt, st4 * P : (st4 + 1) * P],
                                pst[:, dt, :],
                                ln1w_c(dt),
                                ln1b_c(dt),
                                ALU.mult,
                                ALU.add,
                            )
                        else:
                            nc.scalar.activation(
                                out=hTx[:, dt, st4 * P : (st4 + 1) * P],
                                in_=pst[:, dt, :],
                                func=AF.Identity,
                                bias=ln1b_c(dt),
                                scale=ln1w_c(dt),
                            )
                    if st >= 1:
                        emit_v(st - 1, 0)
                    if st >= 2:
                        emit_v(st - 2, 1)
                emit_v(NS - 1, 0)
                emit_v(NS - 2, 1)
                emit_v(NS - 1, 1)

                for ot in range(ND):
                    ps = ps_q.tile([P, SO], F32, tag="ps_q")
                    for kt in range(ND):
                        nc.tensor.matmul(
                            ps[:],
                            Wq_sb[:, kt, ot * P : (ot + 1) * P],
                            hT_own[:, kt, :],
                            start=(kt == 0),
                            stop=(kt == ND - 1),
                        )
                    nc.vector.tensor_scalar(
                        QT[:, ot, :], ps[:], bq_c(ot), None, ALU.add
                    )

            # ------------------- pipeline: K_j projection + attention_j
            with ExitStack() as phC:
                attn = phC.enter_context(tc.tile_pool(name="attn", bufs=1))
                bcp = phC.enter_context(tc.tile_pool(name="bcp", bufs=2))
                prp = phC.enter_context(tc.tile_pool(name="prp", bufs=4))
                ps_k = phC.enter_context(tc.tile_pool(name="ps_k", bufs=1, space="PSUM"))
                ps_s = phC.enter_context(tc.tile_pool(name="ps_s", bufs=2, space="PSUM"))
                ps_o = phC.enter_context(tc.tile_pool(name="ps_o", bufs=1, space="PSUM"))

                def emit_k(j):
                    psk = ps_k.tile([P, 2, SO], F32, tag="ps_k")
                    for sh in range(2):
                        hTx = hT_own if sh == 0 else hT_oth
                        for kt in range(ND):
                            nc.tensor.matmul(
                                psk[:, sh, :],
                                Wk_sb[:, kt, j * P : (j + 1) * P],
                                hTx[:, kt, :],
                                start=(kt == 0),
                                stop=(kt == ND - 1),
                            )
                        nc.vector.tensor_scalar(
                            KT[:, j, sh * SO : (sh + 1) * SO],
                            psk[:, sh, :],
                            bk_c(j),
                            None,
                            ALU.add,
                        )

                rec = attn.tile([P, 2, SO], BF16, name="rec")
                nc.gpsimd.memset(rec[:], 0.0)
                # pre-bias the residual with bo (x + bo), in place; gpsimd so
                # the DVE FIFO never stalls on the xN_own DMA
                for st in range(NSO):
                    nc.gpsimd.tensor_tensor(
                        xN_own[:, st, :], xN_own[:, st, :], bo_bc[:], ALU.add
                    )

                emit_k(0)
                for j in range(ND):
                    if j + 1 < ND:
                        emit_k(j + 1)
                    po = ps_o.tile([P, 2, SO], F32, tag="ps_o")
                    for kb in range(NS):
                        pss = ps_s.tile([P, 2, SO], F32, tag="ps_s")
                        nc.tensor.matmul(
                            pss[:, 0, :],
                            KT[0:DH, j, kb * P : (kb + 1) * P],
                            QT[0:DH, j, :],
                            start=True,
                            stop=True,
                        )
                        nc.tensor.matmul(
                            pss[:, 1, :],
                            KT[DH:P, j, kb * P : (kb + 1) * P],
                            QT[DH:P, j, :],
                            start=True,
                            stop=True,
                        )
                        prob = prp.tile([P, 2, SO], BF16, tag="prob")
                        nc.scalar.activation(
                            out=prob[:], in_=pss[:], func=AF.Exp, scale=0.125
                        )
                        nc.tensor.matmul(
                            po[0:65, 0, :],
                            VN[:, kb, 2 * j, 0:65],
                            prob[:, 0, :],
                            start=(kb == 0),
                            stop=(kb == NS - 1),
                        )
                        nc.tensor.matmul(
                            po[0:65, 1, :],
                            VN[:, kb, 2 * j + 1, 0:65],
                            prob[:, 1, :],
                            start=(kb == 0),
                            stop=(kb == NS - 1),
                        )
                    # normalize: denominators live in psum row DH; 1/Z on
                    # DVE, broadcast over partitions with a zero-padded K=128
                    # matmul vs e0 (into the shared ps_s pool), multiply on
                    # DVE with partition-shifted writes into OT halves.
                    lnZ = attn.tile([1, 2, SO], F32, tag="lnZ")
                    nc.scalar.activation(
                        out=lnZ[:], in_=po[DH : DH + 1, :, :], func=AF.Ln
                    )
                    nc.scalar.activation(
                        out=rec[0:1, :, :], in_=lnZ[:], func=AF.Exp, scale=-1.0
                    )
                    psb = ps_k.tile([P, 2, SO], F32, tag="ps_k")
                    for hh in range(2):
                        nc.tensor.matmul(
                            psb[:, hh, :], e0[:], rec[:, hh, :],
                            start=True, stop=True,
                        )
                    bcast = bcp.tile([DH, 2, SO], F32, tag="bcast")
                    nc.vector.tensor_copy(out=bcast[:], in_=psb[0:DH, :, :])
                    nc.vector.tensor_tensor(
                        OT[0:DH, j, :], po[0:DH, 0, :], bcast[:, 0, :], ALU.mult
                    )
                    nc.vector.tensor_tensor(
                        OT[DH:P, j, :], po[0:DH, 1, :], bcast[:, 1, :], ALU.mult
                    )

            wk_cm.__exit__(None, None, None)
            qkv_cm.__exit__(None, None, None)
            hT_cm.__exit__(None, None, None)

            # ------------------- Wo projection + LN2 + h^T, fused residual
            x1_cm = tc.tile_pool(name="x1p", bufs=1)
            x1p = x1_cm.__enter__()
            x1N = x1p.tile([P, NSO, D], F32, name="x1N")

            with ExitStack() as phD:
                lnp2 = phD.enter_context(tc.tile_pool(name="ln2", bufs=2))
                hn2p = phD.enter_context(tc.tile_pool(name="hn2p", bufs=4))
                psD = phD.enter_context(tc.tile_pool(name="psD", bufs=2, space="PSUM"))
                ps_t2 = phD.enter_context(
                    tc.tile_pool(name="ps_t2", bufs=2, space="PSUM")
                )

                hn2_tiles = {}

                def emit_t2(qb):
                    hn2 = hn2_tiles.pop(qb)
                    pst2 = ps_t2.tile([P, ND, P], BF16, tag="ps_t2")
                    for dt in range(ND):
                        nc.tensor.transpose(
                            pst2[:, dt, :], hn2[:, dt * P : (dt + 1) * P], ident[:]
                        )
                        if dt % 2 == 0:
                            nc.vector.tensor_scalar(
                                h2T[:, dt, qb * P : (qb + 1) * P],
                                pst2[:, dt, :],
                                ln2w_c(dt),
                                ln2b_c(dt),
                                ALU.mult,
                                ALU.add,
                            )
                        else:
                            nc.scalar.activation(
                                out=h2T[:, dt, qb * P : (qb + 1) * P],
                                in_=pst2[:, dt, :],
                                func=AF.Identity,
                                bias=ln2b_c(dt),
                                scale=ln2w_c(dt),
                            )

                for qb in range(NSO):
                    for dh in range(2):
                        ps = psD.tile([P, 512], F32, tag="ps_d")
                        for kt in range(ND):
                            nc.tensor.matmul(
                                ps[:],
                                OT[:, kt, qb * P : (qb + 1) * P],
                                Wo_sb[:, kt, dh * 512 : (dh + 1) * 512],
                                start=(kt == 0),
                                stop=(kt == ND - 1),
                            )
                        nc.vector.tensor_tensor(
                            x1N[:, qb, dh * 512 : (dh + 1) * 512],
                            xN_own[:, qb, dh * 512 : (dh + 1) * 512],
                            ps[:],
                            ALU.add,
                        )
                    # LN2 chain for this row-block starts while the next
                    # blocks' Wo matmuls keep the PE busy; all h^T
                    # transposes run at the end.
                    hn2 = hn2p.tile([P, D], BF16, tag="hn2")
                    ln_tile(lnp2, x1N[:, qb, :], hn2[:], eps_t, "l2")
                    nc.vector.tensor_tensor(
                        x1N[:, qb, :], x1N[:, qb, :], bp_bc[:], ALU.add
                    )
                    hn2_tiles[qb] = hn2
                for qb in range(NSO):
                    emit_t2(qb)

            # ----------------------------------------------- MLP
            with ExitStack() as phF:
                gtp = phF.enter_context(tc.tile_pool(name="gtp", bufs=1))
                wpp = phF.enter_context(tc.tile_pool(name="wpp", bufs=1))
                wcst = phF.enter_context(tc.tile_pool(name="wcst", bufs=4))
                psF = phF.enter_context(tc.tile_pool(name="psF", bufs=2, space="PSUM"))
                psP = phF.enter_context(tc.tile_pool(name="psP", bufs=2, space="PSUM"))
                opool = phF.enter_context(tc.tile_pool(name="opool", bufs=3))

                GT = gtp.tile([P, NF, SO], BF16, name="GT")
                Wp_sb = wpp.tile([P, NF, D], BF16, name="Wp_sb")

                for ft in range(NF):
                    wfc = wcst.tile([P, ND, P], BF16, tag="wfc")
                    nc.sync.dma_start(
                        out=wfc[:],
                        in_=wfc_ext[:, ft * D : (ft + 1) * D].rearrange(
                            "p (k f) -> p k f", k=ND
                        ),
                    )
                    nc.scalar.dma_start(
                        out=Wp_sb[:, ft, :], in_=wp_ext[:, ft * D : (ft + 1) * D]
                    )
                    ps = psF.tile([P, SO], F32, tag="ps_g")
                    for kt in range(ND):
                        nc.tensor.matmul(
                            ps[:],
                            wfc[:, kt, :],
                            h2T[:, kt, :],
                            start=(kt == 0),
                            stop=(kt == ND - 1),
                        )
                    nc.scalar.activation(
                        out=GT[:, ft, :],
                        in_=ps[:],
                        func=AF.Gelu,
                        bias=bfc_c(ft),
                    )

                # proj, NATURAL output, fused residual:
                # out[s, d] = (x1 + bproj)[s, d] + sum_ft GT[:,ft,s].T @ Wp[ft, d]
                for qb in range(NSO):
                    for dh in range(2):
                        ps = psP.tile([P, 512], F32, tag="ps_p")
                        for ft in range(NF):
                            nc.tensor.matmul(
                                ps[:],
                                GT[:, ft, qb * P : (qb + 1) * P],
                                Wp_sb[:, ft, dh * 512 : (dh + 1) * 512],
                                start=(ft == 0),
                                stop=(ft == NF - 1),
                            )
                        of = opool.tile([P, 512], F32, tag="of")
                        nc.vector.tensor_tensor(
                            of[:],
                            x1N[:, qb, dh * 512 : (dh + 1) * 512],
                            ps[:],
                            ALU.add,
                        )
                        nc.scalar.dma_start(
                            out=out_ext[qb * P : (qb + 1) * P, dh * 512 : (dh + 1) * 512],
                            in_=of[:],
                        )

            x1_cm.__exit__(None, None, None)
            mid_cm.__exit__(None, None, None)

    _split_multiwaits(nc)
    return nc


_NC_CACHE = None


def _get_nc():
    global _NC_CACHE
    if _NC_CACHE is None:
        _NC_CACHE = build()
    return _NC_CACHE


def _pack_weights(inputs):
    """Host-side: cast weights to bf16 and pre-arrange into SBUF layouts."""
    import ml_dtypes

    bf = ml_dtypes.bfloat16
    Wq = np.asarray(inputs["Wq"], np.float32)
    Wk = np.asarray(inputs["Wk"], np.float32)
    Wv = np.asarray(inputs["Wv"], np.float32)
    Wo = np.asarray(inputs["Wo"], np.float32)
    Wfc = np.asarray(inputs["Wfc"], np.float32)
    Wp = np.asarray(inputs["Wproj"], np.float32)

    def pack_dd(W):  # [D, D] -> [P, ND*D], [p, kt*D+c] = W[kt*P+p, c]
        return np.ascontiguousarray(
            W.reshape(ND, P, D).transpose(1, 0, 2).reshape(P, ND * D).astype(bf)
        )

    wq_p = pack_dd(Wq)
    wk_p = pack_dd(Wk)
    wo_p = pack_dd(Wo)
    # [p, oh*ND*SO + kt*SO + c] = Wv[kt*P+p, oh*SO+c]
    wv_p = np.ascontiguousarray(
        Wv.reshape(ND, P, 2, SO).transpose(1, 2, 0, 3).reshape(P, 2 * ND * SO).astype(bf)
    )
    # [p, ft*ND*P + kt*P + f] = Wfc[kt*P+p, ft*P+f]
    wfc_p = np.ascontiguousarray(
        Wfc.reshape(ND, P, NF, P).transpose(1, 2, 0, 3).reshape(P, NF * ND * P).astype(bf)
    )
    # [p, ft*D + c] = Wp[ft*P+p, c]
    wp_p = np.ascontiguousarray(
        Wp.reshape(NF, P, D).transpose(1, 0, 2).reshape(P, NF * D).astype(bf)
    )
    return {
        "wq_p": wq_p, "wk_p": wk_p, "wv_p": wv_p, "wo_p": wo_p,
        "wfc_p": wfc_p, "wp_p": wp_p,
    }


def _pack_consts(inputs):
    """Host-side: pack all small vectors into one [128, NCONST] f32 block."""
    def vec(n):
        return np.asarray(inputs[n], np.float32)

    blk = np.zeros((P, NCONST), np.float32)
    blk[:, OFF_LN1W : OFF_LN1W + ND] = vec("ln1_w").reshape(ND, P).T
    blk[:, OFF_LN1B : OFF_LN1B + ND] = vec("ln1_b").reshape(ND, P).T
    blk[:, OFF_LN2W : OFF_LN2W + ND] = vec("ln2_w").reshape(ND, P).T
    blk[:, OFF_LN2B : OFF_LN2B + ND] = vec("ln2_b").reshape(ND, P).T
    blk[:, OFF_BQ : OFF_BQ + ND] = vec("bq").reshape(ND, P).T
    blk[:, OFF_BK : OFF_BK + ND] = vec("bk").reshape(ND, P).T
    blk[:, OFF_BFC : OFF_BFC + NF] = vec("bfc").reshape(NF, P).T
    blk[:, OFF_BV : OFF_BV + D] = vec("bv")[None, :]
    blk[:, OFF_BO : OFF_BO + D] = vec("bo")[None, :]
    blk[:, OFF_BP : OFF_BP + D] = vec("bproj")[None, :]
    return np.ascontiguousarray(blk)


def make_in_maps(inputs):
    """Shard FULL inputs into per-core input maps (own rows rotated first)."""
    import ml_dtypes

    bf = ml_dtypes.bfloat16
    x = np.asarray(inputs["x"], dtype=np.float32)
    shared = {"consts_p": _pack_consts(inputs)}
    shared.update(_pack_weights(inputs))
    in_maps = []
    for c in range(N_CORES):
        b, half = c // 2, c % 2
        xb = x[b]
        x_core = np.concatenate(
            [xb[half * SO : (half + 1) * SO], xb[(1 - half) * SO : (2 - half) * SO]],
            axis=0,
        )
        m = {
            "x_bf": np.ascontiguousarray(x_core.astype(bf)),
            "x_own": np.ascontiguousarray(x_core[0:SO]),
        }
        m.update(shared)
        in_maps.append(m)
    return in_maps


def kernel(**inputs) -> np.ndarray:
    from concourse.bass_utils import run_bass_kernel_spmd

    nc = _get_nc()
    in_maps = make_in_maps(inputs)
    res = run_bass_kernel_spmd(nc, in_maps, list(range(N_CORES)))
    B = 4
    out = np.empty((B, S, D), dtype=np.float32)
    for c in range(N_CORES):
        b, half = c // 2, c % 2
        out[b, half * SO : (half + 1) * SO] = res.results[c]["out"]
    return out


# revision 22
# speedup vs baseline: 1.0651x; 1.0651x over previous
"""Trainium2 Bass kernel for a dense transformer block (B=4, N=1024, D=1024,
H=16, Dh=64, MLP 4x), distributed over 8 NeuronCores with ZERO collectives.

Sharding: core c handles batch b = c//2, sequence half = c%2 (512 query
rows).  K/V are computed for the batch's full 1024-token sequence on both
cores of a pair; the sequence is rotated per-core so the core's own 512 rows
are always rows 0..511 of its input, so all 8 cores run one identical SPMD
program.

v2 layout/scheduling notes:
- All weights are cast to bf16 AND pre-packed into their exact SBUF layouts
  on the HOST, so every weight DMA is a contiguous [128, N] blast and no
  on-chip casting is needed (v1 spent ~200us of engine time on casts and
  streamed 48MB of f32 weights through one DMA queue).
- x is passed twice: full-sequence bf16 (feeds LN1) and own-half f32 (the
  residual).  LN1 stats tolerate bf16 input easily at the 2e-2 gate.
- V (then Q) matmuls are interleaved into the LN1 loop so the PE is busy
  from ~5us on; K projection is pipelined per-head-pair against the
  attention inner loop so the ACT-bound softmax EXP (~0.7us per [128,512]
  tile, the real attention bottleneck) overlaps PE matmul work.
- Scores contract over Dh=64: the two heads of a d-tile run as two
  CONCURRENT K=64 matmuls in distinct row-groups of the PE array
  (tile_position (0,0)/(64,0)), halving score PE time.
- Softmax denominators ride in column 64 of V (ones column); the
  normalization reciprocal-broadcast runs on ACT+GPSIMD+DVE, not PE.
"""

import numpy as np

import bass_rust
import concourse.bass as bass
import concourse.mybir as mybir
import concourse.tile as tile
from concourse import library_config
from concourse.masks import make_identity

F32 = mybir.dt.float32
BF16 = mybir.dt.bfloat16
AF = mybir.ActivationFunctionType
ALU = mybir.AluOpType

P = 128
D = 1024
S = 1024          # full sequence (per batch)
SO = 512          # own rows per core
H = 16
DH = 64
F = 4096
EPS = 1e-5
N_CORES = 8

ND = D // P       # 8   d tiles
NS = S // P       # 8   full-seq tiles
NSO = SO // P     # 4   own-seq tiles
NF = F // P       # 32  ff tiles

# packed-consts block column offsets
OFF_LN1W, OFF_LN1B, OFF_LN2W, OFF_LN2B = 0, 8, 16, 24
OFF_BQ, OFF_BK, OFF_BFC = 32, 40, 48
OFF_BV, OFF_BO, OFF_BP = 80, 80 + 1024, 80 + 2048
NCONST = 80 + 3 * 1024


# --------------------------------------------------------------------------
# Workaround: this compiler build supports only ONE semaphore wait per
# instruction.  Move excess waits onto fresh NOPs inserted just before the
# offending instruction on the same engine.
# --------------------------------------------------------------------------
_counter = [0]


def _split_multiwaits(nc):
    nsplit = 0
    for fn in nc.m.functions:
        for blk in fn.blocks:
            il = list(blk.instructions)
            out = []
            changed = False
            for inst in il:
                si = inst.sync_info
                if si is not None and len(si.on_wait) > 1:
                    waits = list(si.on_wait)
                    for w in waits[:-1]:
                        _counter[0] += 1
                        nop = mybir.InstNoOp(
                            name=f"I-waitsplit-{_counter[0]}", ins=[], outs=[]
                        )
                        nop.engine = inst.engine
                        nop.sync_info = bass_rust.SyncInfo(on_wait=[w], on_update=[])
                        out.append(nop)
                        nc.register_instruction(nop, overwrite=True)
                    inst.sync_info = bass_rust.SyncInfo(
                        on_wait=[waits[-1]], on_update=list(si.on_update)
                    )
                    changed = True
                    nsplit += 1
                out.append(inst)
            if changed:
                blk.instructions = out
    return nsplit


def _vec_tile(nc, pool, ext, n):
    """Load a [n*128] dram vector as a [128, n] sbuf tile (col i = tile i)."""
    t = pool.tile([P, n], F32, name=ext.name + "_sb")
    nc.scalar.dma_start(out=t[:], in_=ext[:].rearrange("(o p) -> p o", p=P))
    return t


def _bcast_tile(nc, pool, ext, n):
    """Load a [n] dram vector broadcast to a [128, n] sbuf tile."""
    t = pool.tile([P, n], F32, name=ext.name + "_bc")
    ap = ext[:]
    src = bass.AP(tensor=ap.tensor, offset=ap.offset, ap=[[0, P], ap.ap[0]])
    nc.scalar.dma_start(out=t[:], in_=src)
    return t


def build():
    nc = bass.Bass(name="tfblock")

    xbf_ext = nc.declare_dram_parameter("x_bf", [S, D], BF16, isOutput=False)
    xo_ext = nc.declare_dram_parameter("x_own", [SO, D], F32, isOutput=False)
    wq_ext = nc.declare_dram_parameter("wq_p", [P, ND * D], BF16, isOutput=False)
    wk_ext = nc.declare_dram_parameter("wk_p", [P, ND * D], BF16, isOutput=False)
    wv_ext = nc.declare_dram_parameter("wv_p", [P, 2 * ND * SO], BF16, isOutput=False)
    wo_ext = nc.declare_dram_parameter("wo_p", [P, ND * D], BF16, isOutput=False)
    wfc_ext = nc.declare_dram_parameter("wfc_p", [P, NF * ND * P], BF16, isOutput=False)
    wp_ext = nc.declare_dram_parameter("wp_p", [P, NF * D], BF16, isOutput=False)
    cp_ext = nc.declare_dram_parameter("consts_p", [P, NCONST], F32, isOutput=False)
    out_ext = nc.declare_dram_parameter("out", [SO, D], F32, isOutput=True)

    def ln_tile(lnp, src_ap, hn_out, eps_t, tag):
        """LayerNorm stats on DVE + apply on ACT: hn_out = (src-mu)*rstd."""
        stats = lnp.tile([P, 2, 6], F32, tag=tag + "_st")
        for g in range(2):
            nc.vector.bn_stats(out=stats[:, g, :], in_=src_ap[:, g * 512 : (g + 1) * 512])
        mv = lnp.tile([P, 2], F32, tag=tag + "_mv")
        nc.vector.bn_aggr(out=mv[:], in_=stats[:])
        lnv = lnp.tile([P, 1], F32, tag=tag + "_sd")
        nc.scalar.activation(out=lnv[:], in_=mv[:, 1:2], func=AF.Ln, bias=eps_t[:])
        rstd = lnp.tile([P, 1], F32, tag=tag + "_rs")
        nc.scalar.activation(out=rstd[:], in_=lnv[:], func=AF.Exp, scale=-0.5)
        nb = lnp.tile([P, 1], F32, tag=tag + "_nb")
        nc.vector.tensor_scalar(nb[:], mv[:, 0:1], rstd[:], -1.0, ALU.mult, ALU.mult)
        nc.scalar.activation(
            out=hn_out, in_=src_ap, func=AF.Identity, bias=nb[:], scale=rstd[:]
        )

    with tile.TileContext(nc) as tc:
        from contextlib import ExitStack

        with ExitStack() as top:
            consts = top.enter_context(tc.tile_pool(name="consts", bufs=1))
            persist = top.enter_context(tc.tile_pool(name="persist", bufs=1))

            # all small consts arrive as ONE host-packed [128, 3152] f32
            # block on the ACT hwdge queue (tiny strided DMAs previously
            # jammed that queue for ~45us)
            cblk = consts.tile([P, NCONST], F32, name="cblk")
            def _col(off):
                return lambda i: cblk[:, off + i : off + i + 1]

            ln1w_c = _col(OFF_LN1W)
            ln1b_c = _col(OFF_LN1B)
            ln2w_c = _col(OFF_LN2W)
            ln2b_c = _col(OFF_LN2B)
            bq_c = _col(OFF_BQ)
            bk_c = _col(OFF_BK)
            bfc_c = _col(OFF_BFC)

            def bv_sl(a, b):
                return cblk[:, OFF_BV + a : OFF_BV + b]

            bo_bc = cblk[:, OFF_BO : OFF_BO + D]
            bp_bc = cblk[:, OFF_BP : OFF_BP + D]

            eps_t = consts.tile([P, 1], F32, name="eps")
            nc.vector.memset(eps_t[:], EPS)
            ident = consts.tile([P, P], BF16, name="ident")
            make_identity(nc, ident[:])
            e0 = consts.tile([P, P], BF16, name="e0")
            nc.vector.memset(e0[:], 0.0)
            nc.vector.memset(e0[0:1, :], 1.0)

            # residual stream (own half), f32, on the ACT queue (off the
            # critical startup path of the sync queue)
            xN_own = persist.tile([P, NSO, D], F32, name="xN_own")

            # long-lived mid tensors: Wo (consumed in the Wo projection),
            # OT (attention out, transposed), h2T (LN2 out, transposed)
            mid_cm = tc.tile_pool(name="mid", bufs=1)
            midp = mid_cm.__enter__()
            Wo_sb = midp.tile([P, ND, D], BF16, name="Wo_sb")
            OT = midp.tile([P, ND, SO], BF16, name="OT")
            h2T = midp.tile([P, ND, SO], BF16, name="h2T")

            # transposed-LN1 outputs + QKV live until end of attention
            hT_cm = tc.tile_pool(name="hTp", bufs=1)
            hTp = hT_cm.__enter__()
            hT_own = hTp.tile([P, ND, SO], BF16, name="hT_own")
            hT_oth = hTp.tile([P, ND, SO], BF16, name="hT_oth")

            qkv_cm = tc.tile_pool(name="qkvp", bufs=1)
            qkvp = qkv_cm.__enter__()
            QT = qkvp.tile([P, ND, SO], BF16, name="QT")
            KT = qkvp.tile([P, ND, S], BF16, name="KT")
            VN = qkvp.tile([P, NS, H, 66], BF16, name="VN")
            nc.vector.memset(VN[:, :, :, DH : DH + 1], 1.0)

            wk_cm = tc.tile_pool(name="wkp", bufs=1)
            wkp = wk_cm.__enter__()
            Wk_sb = wkp.tile([P, ND, D], BF16, name="Wk_sb")

            # ------------------------- phase B: x + W DMAs, LN1, V, Q
            with ExitStack() as phB:
                wv_pool = phB.enter_context(tc.tile_pool(name="wvp", bufs=1))
                xbf_pool = phB.enter_context(tc.tile_pool(name="xbfp", bufs=1))
                hnp = phB.enter_context(tc.tile_pool(name="hnp", bufs=3))
                lnp = phB.enter_context(tc.tile_pool(name="ln1", bufs=2))
                ps_t = phB.enter_context(tc.tile_pool(name="ps_t", bufs=2, space="PSUM"))
                ps_v = phB.enter_context(tc.tile_pool(name="ps_v", bufs=2, space="PSUM"))
                ps_q = phB.enter_context(tc.tile_pool(name="ps_q", bufs=2, space="PSUM"))

                Wv_sb = wv_pool.tile([P, 2, ND, SO], BF16, name="Wv_sb")
                Wq_sb = wv_pool.tile([P, ND, D], BF16, name="Wq_sb")
                xbf = xbf_pool.tile([P, NS, D], BF16, name="xbf")

                # sync-queue DMA order is the startup schedule: x tiles for
                # the own half, first V weight half, rest of x, V second
                # half, then Wq/Wk/Wo.
                # ONE fast queue, explicitly ordered: ln consts, own-half x,
                # remaining consts, Wv halves, other-half x, Wq, Wk, Wo.
                nc.sync.dma_start(out=cblk[:, 0:OFF_BV], in_=cp_ext[:, 0:OFF_BV])
                for st in range(4):
                    nc.sync.dma_start(
                        out=xbf[:, st, :], in_=xbf_ext[st * P : (st + 1) * P, :]
                    )
                nc.sync.dma_start(
                    out=cblk[:, OFF_BV:], in_=cp_ext[:, OFF_BV:]
                )
                nc.sync.dma_start(
                    out=Wv_sb[:, 0, :, :],
                    in_=wv_ext[:, 0 : ND * SO].rearrange("p (k c) -> p k c", k=ND),
                )
                nc.sync.dma_start(
                    out=Wv_sb[:, 1, :, :],
                    in_=wv_ext[:, ND * SO :].rearrange("p (k c) -> p k c", k=ND),
                )
                for st in range(4, NS):
                    nc.sync.dma_start(
                        out=xbf[:, st, :], in_=xbf_ext[st * P : (st + 1) * P, :]
                    )
                nc.sync.dma_start(
                    out=Wq_sb[:], in_=wq_ext[:].rearrange("p (k c) -> p k c", k=ND)
                )
                nc.sync.dma_start(
                    out=Wk_sb[:], in_=wk_ext[:].rearrange("p (k c) -> p k c", k=ND)
                )
                nc.sync.dma_start(
                    out=Wo_sb[:], in_=wo_ext[:].rearrange("p (k c) -> p k c", k=ND)
                )
                nc.scalar.dma_start(
                    out=xN_own[:], in_=xo_ext[:].rearrange("(t p) d -> p t d", p=P)
                )

                def emit_v(st, oh):
                    hTx = hT_own if st < 4 else hT_oth
                    st4 = st % 4
                    ps = ps_v.tile([P, SO], F32, tag="ps_v")
                    for kt in range(ND):
                        nc.tensor.matmul(
                            ps[:],
                            hTx[:, kt, st4 * P : (st4 + 1) * P],
                            Wv_sb[:, oh, kt, :],
                            start=(kt == 0),
                            stop=(kt == ND - 1),
                        )
                    nc.vector.tensor_tensor(
                        VN[:, st, oh * 8 : (oh + 1) * 8, 0:DH],
                        ps[:].rearrange("p (h e) -> p h e", h=8),
                        bv_sl(oh * 512, (oh + 1) * 512).rearrange(
                            "p (h e) -> p h e", h=8
                        ),
                        ALU.add,
                    )

                # LN1 per tile; h^T PE-transposes (ln1 w/b fused in the DVE
                # copy-back) run one tile ahead of the V matmuls
                for st in range(NS):
                    hn = hnp.tile([P, D], BF16, tag="hn")
                    ln_tile(lnp, xbf[:, st, :], hn[:], eps_t, "l1")
                    hTx = hT_own if st < 4 else hT_oth
                    st4 = st % 4
                    pst = ps_t.tile([P, ND, P], BF16, tag="ps_t")
                    for dt in range(ND):
                        nc.tensor.transpose(
                            pst[:, dt, :], hn[:, dt * P : (dt + 1) * P], ident[:]
                        )
                        if dt % 2 == 0:
                            nc.vector.tensor_scalar(
                                hTx[:, dt, st4 * P : (st4 + 1) * P],
                                pst[:, dt, :],
                                ln1w_c(dt),
                                ln1b_c(dt),
                                ALU.mult,
                                ALU.add,
                            )
                        else:
                            nc.scalar.activation(
                                out=hTx[:, dt, st4 * P : (st4 + 1) * P],
                                in_=pst[:, dt, :],
                                func=AF.Identity,
                                bias=ln1b_c(dt),
                                scale=ln1w_c(dt),
                            )
                    if st >= 1:
                        emit_v(st - 1, 0)
                    if st >= 2:
                        emit_v(st - 2, 1)
                emit_v(NS - 1, 0)
                emit_v(NS - 2, 1)
                emit_v(NS - 1, 1)

                for ot in range(ND):
                    ps = ps_q.tile([P, SO], F32, tag="ps_q")
                    for kt in range(ND):
                        nc.tensor.matmul(
                            ps[:],
                            Wq_sb[:, kt, ot * P : (ot + 1) * P],
                            hT_own[:, kt, :],
                            start=(kt == 0),
                            stop=(kt == ND - 1),
                        )
                    nc.vector.tensor_scalar(
                        QT[:, ot, :], ps[:], bq_c(ot), None, ALU.add
                    )

            # ------------------- pipeline: K_j projection + attention_j
            with ExitStack() as phC:
                attn = phC.enter_context(tc.tile_pool(name="attn", bufs=1))
                bcp = phC.enter_context(tc.tile_pool(name="bcp", bufs=2))
                prp = phC.enter_context(tc.tile_pool(name="prp", bufs=4))
                ps_k = phC.enter_context(tc.tile_pool(name="ps_k", bufs=1, space="PSUM"))
                ps_s = phC.enter_context(tc.tile_pool(name="ps_s", bufs=2, space="PSUM"))
                ps_o = phC.enter_context(tc.tile_pool(name="ps_o", bufs=1, space="PSUM"))

                def emit_k(j):
                    psk = ps_k.tile([P, 2, SO], F32, tag="ps_k")
                    for sh in range(2):
                        hTx = hT_own if sh == 0 else hT_oth
                        for kt in range(ND):
                            nc.tensor.matmul(
                                psk[:, sh, :],
                                Wk_sb[:, kt, j * P : (j + 1) * P],
                                hTx[:, kt, :],
                                start=(kt == 0),
                                stop=(kt == ND - 1),
                            )
                        nc.vector.tensor_scalar(
                            KT[:, j, sh * SO : (sh + 1) * SO],
                            psk[:, sh, :],
                            bk_c(j),
                            None,
                            ALU.add,
                        )

                rec = attn.tile([P, 2, SO], BF16, name="rec")
                nc.gpsimd.memset(rec[:], 0.0)
                # pre-bias the residual with bo (x + bo), in place; gpsimd so
                # the DVE FIFO never stalls on the xN_own DMA
                for st in range(NSO):
                    nc.gpsimd.tensor_tensor(
                        xN_own[:, st, :], xN_own[:, st, :], bo_bc[:], ALU.add
                    )

                emit_k(0)
                for j in range(ND):
                    if j + 1 < ND:
                        emit_k(j + 1)
                    po = ps_o.tile([P, 2, SO], F32, tag="ps_o")
                    for kb in range(NS):
                        pss = ps_s.tile([P, 2, SO], F32, tag="ps_s")
                        nc.tensor.matmul(
                            pss[:, 0, :],
                            KT[0:DH, j, kb * P : (kb + 1) * P],
                            QT[0:DH, j, :],
                            start=True,
                            stop=True,
                        )
                        nc.tensor.matmul(
                            pss[:, 1, :],
                            KT[DH:P, j, kb * P : (kb + 1) * P],
                            QT[DH:P, j, :],
                            start=True,
                            stop=True,
                        )
                        prob = prp.tile([P, 2, SO], BF16, tag="prob")
                        nc.scalar.activation(
                            out=prob[:], in_=pss[:], func=AF.Exp, scale=0.125
                        )
                        nc.tensor.matmul(
                            po[0:65, 0, :],
                            VN[:, kb, 2 * j, 0:65],
                            prob[:, 0, :],
                            start=(kb == 0),
                            stop=(kb == NS - 1),
                        )
                        nc.tensor.matmul(
                            po[0:65, 1, :],
                            VN[:, kb, 2 * j + 1, 0:65],
                            prob[:, 1, :],
                            start=(kb == 0),
                            stop=(kb == NS - 1),
                        )
                    # normalize: denominators live in psum row DH; 1/Z on
                    # DVE, broadcast over partitions with a zero-padded K=128
                    # matmul vs e0 (into the shared ps_s pool), multiply on
                    # DVE with partition-shifted writes into OT halves.
                    lnZ = attn.tile([1, 2, SO], F32, tag="lnZ")
                    nc.scalar.activation(
                        out=lnZ[:], in_=po[DH : DH + 1, :, :], func=AF.Ln
                    )
                    nc.scalar.activation(
                        out=rec[0:1, :, :], in_=lnZ[:], func=AF.Exp, scale=-1.0
                    )
                    # broadcast 1/Z into po's UNUSED partitions 64..127 (the
                    # AV out only fills 0..64) via col-group tiling
                    for hh in range(2):
                        nc.tensor.matmul(
                            po[DH:P, hh, :], e0[:, 0:DH], rec[:, hh, :],
                            start=True, stop=True, skip_group_check=True,
                        )
                    bcast = bcp.tile([DH, 2, SO], F32, tag="bcast")
                    nc.vector.tensor_copy(out=bcast[:], in_=po[DH:P, :, :])
                    nc.vector.tensor_tensor(
                        OT[0:DH, j, :], po[0:DH, 0, :], bcast[:, 0, :], ALU.mult
                    )
                    nc.vector.tensor_tensor(
                        OT[DH:P, j, :], po[0:DH, 1, :], bcast[:, 1, :], ALU.mult
                    )

            wk_cm.__exit__(None, None, None)
            qkv_cm.__exit__(None, None, None)
            hT_cm.__exit__(None, None, None)

            # ------------------- Wo projection + LN2 + h^T, fused residual
            x1_cm = tc.tile_pool(name="x1p", bufs=1)
            x1p = x1_cm.__enter__()
            x1N = x1p.tile([P, NSO, D], F32, name="x1N")

            with ExitStack() as phD:
                lnp2 = phD.enter_context(tc.tile_pool(name="ln2", bufs=2))
                hn2p = phD.enter_context(tc.tile_pool(name="hn2p", bufs=4))
                psD = phD.enter_context(tc.tile_pool(name="psD", bufs=2, space="PSUM"))
                ps_t2 = phD.enter_context(
                    tc.tile_pool(name="ps_t2", bufs=2, space="PSUM")
                )

                hn2_tiles = {}

                def emit_t2(qb):
                    hn2 = hn2_tiles.pop(qb)
                    pst2 = ps_t2.tile([P, ND, P], BF16, tag="ps_t2")
                    for dt in range(ND):
                        nc.tensor.transpose(
                            pst2[:, dt, :], hn2[:, dt * P : (dt + 1) * P], ident[:]
                        )
                        if dt % 2 == 0:
                            nc.vector.tensor_scalar(
                                h2T[:, dt, qb * P : (qb + 1) * P],
                                pst2[:, dt, :],
                                ln2w_c(dt),
                                ln2b_c(dt),
                                ALU.mult,
                                ALU.add,
                            )
                        else:
                            nc.scalar.activation(
                                out=h2T[:, dt, qb * P : (qb + 1) * P],
                                in_=pst2[:, dt, :],
                                func=AF.Identity,
                                bias=ln2b_c(dt),
                                scale=ln2w_c(dt),
                            )

                for qb in range(NSO):
                    for dh in range(2):
                        ps = psD.tile([P, 512], F32, tag="ps_d")
                        for kt in range(ND):
                            nc.tensor.matmul(
                                ps[:],
                                OT[:, kt, qb * P : (qb + 1) * P],
                                Wo_sb[:, kt, dh * 512 : (dh + 1) * 512],
                                start=(kt == 0),
                                stop=(kt == ND - 1),
                            )
                        nc.vector.tensor_tensor(
                            x1N[:, qb, dh * 512 : (dh + 1) * 512],
                            xN_own[:, qb, dh * 512 : (dh + 1) * 512],
                            ps[:],
                            ALU.add,
                        )
                    # LN2 chain for this row-block starts while the next
                    # blocks' Wo matmuls keep the PE busy; all h^T
                    # transposes run at the end.
                    hn2 = hn2p.tile([P, D], BF16, tag="hn2")
                    ln_tile(lnp2, x1N[:, qb, :], hn2[:], eps_t, "l2")
                    nc.vector.tensor_tensor(
                        x1N[:, qb, :], x1N[:, qb, :], bp_bc[:], ALU.add
                    )
                    hn2_tiles[qb] = hn2
                for qb in range(NSO):
                    emit_t2(qb)

            # ----------------------------------------------- MLP
            with ExitStack() as phF:
                gtp = phF.enter_context(tc.tile_pool(name="gtp", bufs=1))
                wpp = phF.enter_context(tc.tile_pool(name="wpp", bufs=1))
                wcst = phF.enter_context(tc.tile_pool(name="wcst", bufs=4))
                psF = phF.enter_context(tc.tile_pool(name="psF", bufs=2, space="PSUM"))
                psP = phF.enter_context(tc.tile_pool(name="psP", bufs=2, space="PSUM"))
                opool = phF.enter_context(tc.tile_pool(name="opool", bufs=3))

                GT = gtp.tile([P, NF, SO], BF16, name="GT")
                Wp_sb = wpp.tile([P, NF, D], BF16, name="Wp_sb")

                for ft in range(NF):
                    wfc = wcst.tile([P, ND, P], BF16, tag="wfc")
                    nc.sync.dma_start(
                        out=wfc[:],
                        in_=wfc_ext[:, ft * D : (ft + 1) * D].rearrange(
                            "p (k f) -> p k f", k=ND
                        ),
                    )
                    nc.scalar.dma_start(
                        out=Wp_sb[:, ft, :], in_=wp_ext[:, ft * D : (ft + 1) * D]
                    )
                    ps = psF.tile([P, SO], F32, tag="ps_g")
                    for kt in range(ND):
                        nc.tensor.matmul(
                            ps[:],
                            wfc[:, kt, :],
                            h2T[:, kt, :],
                            start=(kt == 0),
                            stop=(kt == ND - 1),
                        )
                    nc.scalar.activation(
                        out=GT[:, ft, :],
                        in_=ps[:],
                        func=AF.Gelu,
                        bias=bfc_c(ft),
                    )

                # proj, NATURAL output, fused residual:
                # out[s, d] = (x1 + bproj)[s, d] + sum_ft GT[:,ft,s].T @ Wp[ft, d]
                for qb in range(NSO):
                    for dh in range(2):
                        ps = psP.tile([P, 512], F32, tag="ps_p")
                        for ft in range(NF):
                            nc.tensor.matmul(
                                ps[:],
                                GT[:, ft, qb * P : (qb + 1) * P],
                                Wp_sb[:, ft, dh * 512 : (dh + 1) * 512],
                                start=(ft == 0),
                                stop=(ft == NF - 1),
                            )
                        of = opool.tile([P, 512], F32, tag="of")
                        nc.vector.tensor_tensor(
                            of[:],
                            x1N[:, qb, dh * 512 : (dh + 1) * 512],
                            ps[:],
                            ALU.add,
                        )
                        nc.scalar.dma_start(
                            out=out_ext[qb * P : (qb + 1) * P, dh * 512 : (dh + 1) * 512],
                            in_=of[:],
                        )

            x1_cm.__exit__(None, None, None)
            mid_cm.__exit__(None, None, None)

    _split_multiwaits(nc)
    return nc


_NC_CACHE = None


def _get_nc():
    global _NC_CACHE
    if _NC_CACHE is None:
        _NC_CACHE = build()
    return _NC_CACHE


def _pack_weights(inputs):
    """Host-side: cast weights to bf16 and pre-arrange into SBUF layouts."""
    import ml_dtypes

    bf = ml_dtypes.bfloat16
    Wq = np.asarray(inputs["Wq"], np.float32)
    Wk = np.asarray(inputs["Wk"], np.float32)
    Wv = np.asarray(inputs["Wv"], np.float32)
    Wo = np.asarray(inputs["Wo"], np.float32)
    Wfc = np.asarray(inputs["Wfc"], np.float32)
    Wp = np.asarray(inputs["Wproj"], np.float32)

    def pack_dd(W):  # [D, D] -> [P, ND*D], [p, kt*D+c] = W[kt*P+p, c]
        return np.ascontiguousarray(
            W.reshape(ND, P, D).transpose(1, 0, 2).reshape(P, ND * D).astype(bf)
        )

    wq_p = pack_dd(Wq)
    wk_p = pack_dd(Wk)
    wo_p = pack_dd(Wo)
    # [p, oh*ND*SO + kt*SO + c] = Wv[kt*P+p, oh*SO+c]
    wv_p = np.ascontiguousarray(
        Wv.reshape(ND, P, 2, SO).transpose(1, 2, 0, 3).reshape(P, 2 * ND * SO).astype(bf)
    )
    # [p, ft*ND*P + kt*P + f] = Wfc[kt*P+p, ft*P+f]
    wfc_p = np.ascontiguousarray(
        Wfc.reshape(ND, P, NF, P).transpose(1, 2, 0, 3).reshape(P, NF * ND * P).astype(bf)
    )
    # [p, ft*D + c] = Wp[ft*P+p, c]
    wp_p = np.ascontiguousarray(
        Wp.reshape(NF, P, D).transpose(1, 0, 2).reshape(P, NF * D).astype(bf)
    )
    return {
        "wq_p": wq_p, "wk_p": wk_p, "wv_p": wv_p, "wo_p": wo_p,
        "wfc_p": wfc_p, "wp_p": wp_p,
    }


def _pack_consts(inputs):
    """Host-side: pack all small vectors into one [128, NCONST] f32 block."""
    def vec(n):
        return np.asarray(inputs[n], np.float32)

    blk = np.zeros((P, NCONST), np.float32)
    blk[:, OFF_LN1W : OFF_LN1W + ND] = vec("ln1_w").reshape(ND, P).T
    blk[:, OFF_LN1B : OFF_LN1B + ND] = vec("ln1_b").reshape(ND, P).T
    blk[:, OFF_LN2W : OFF_LN2W + ND] = vec("ln2_w").reshape(ND, P).T
    blk[:, OFF_LN2B : OFF_LN2B + ND] = vec("ln2_b").reshape(ND, P).T
    blk[:, OFF_BQ : OFF_BQ + ND] = vec("bq").reshape(ND, P).T
    blk[:, OFF_BK : OFF_BK + ND] = vec("bk").reshape(ND, P).T
    blk[:, OFF_BFC : OFF_BFC + NF] = vec("bfc").reshape(NF, P).T
    blk[:, OFF_BV : OFF_BV + D] = vec("bv")[None, :]
    blk[:, OFF_BO : OFF_BO + D] = vec("bo")[None, :]
    blk[:, OFF_BP : OFF_BP + D] = vec("bproj")[None, :]
    return np.ascontiguousarray(blk)


def make_in_maps(inputs):
    """Shard FULL inputs into per-core input maps (own rows rotated first)."""
    import ml_dtypes

    bf = ml_dtypes.bfloat16
    x = np.asarray(inputs["x"], dtype=np.float32)
    shared = {"consts_p": _pack_consts(inputs)}
    shared.update(_pack_weights(inputs))
    in_maps = []
    for c in range(N_CORES):
        b, half = c // 2, c % 2
        xb = x[b]
        x_core = np.concatenate(
            [xb[half * SO : (half + 1) * SO], xb[(1 - half) * SO : (2 - half) * SO]],
            axis=0,
        )
        m = {
            "x_bf": np.ascontiguousarray(x_core.astype(bf)),
            "x_own": np.ascontiguousarray(x_core[0:SO]),
        }
        m.update(shared)
        in_maps.append(m)
    return in_maps


def kernel(**inputs) -> np.ndarray:
    from concourse.bass_utils import run_bass_kernel_spmd

    nc = _get_nc()
    in_maps = make_in_maps(inputs)
    res = run_bass_kernel_spmd(nc, in_maps, list(range(N_CORES)))
    B = 4
    out = np.empty((B, S, D), dtype=np.float32)
    for c in range(N_CORES):
        b, half = c // 2, c % 2
        out[b, half * SO : (half + 1) * SO] = res.results[c]["out"]
    return out
